# revision 2
# baseline (speedup 1.0000x reference)
"""Bahdanau additive cross-attention pooler on 8 TRN2 NeuronCores.

Math (per batch n):
    wq = q @ Ww.T + Wb                      [16, 128]
    uk = k @ Uw.T + Ub                      [2048, 128]
    s[q,k] = V . tanh(wq[q] + uk[k])        [16, 2048]
    w = softmax(s, axis=q)                  (mask and Vb terms are constant in
                                             q, so they cancel in the softmax
                                             exactly -> never computed)
    x[q] = sum_k w[q,k] * k[k]              [16, 128]

Distribution: batch N=32 data-parallel over 8 cores (4 batches/core),
params replicated. No collectives.

Per-core device pipeline (all layouts transposed so d lives on partitions):
    biasq[d, (n,q)] = WwT.T @ qT + Wb + Ub      (one matmul + tensor_scalar)
    per n:
      kT[d, k]   <- DMA-xbar-transpose of bf16 copy of k
      knat[p,kc,d] <- natural k (fp32)
      uk_psum[d, k] = UwT.T @ kT                 (PSUM, 4 matmuls)
      per q: hT = tanh(uk_psum + biasq[:, nq])   (one fused ACT instr, PSUM src)
             s_psum[:, kc*16+q] = hT_chunk.T @ V (16 small matmuls)
      exp_sb = exp(s_psum); Z = rowsum by kc-group; wT = exp_sb * recip(Z)
      x_psum[q, d] = sum_kc wT_chunk.T @ knat_chunk
"""

import sys

sys.path.insert(0, "/opt/trn_rl_repo")

import numpy as np
import ml_dtypes

N_CORES = 8
NB = 4          # batches per core
TQ = 16
TK = 2048
D = 128
KC = TK // 128  # 16 k-chunks of 128

_CACHE = {}


def _build_nc():
    import concourse.bacc as bacc
    import concourse.tile as tile
    from concourse import mybir

    f32 = mybir.dt.float32
    bf16 = mybir.dt.bfloat16
    AF = mybir.ActivationFunctionType

    nc = bacc.Bacc()
    qT_h = nc.declare_dram_parameter("qT", [D, NB * TQ], f32, isOutput=False)
    k_h = nc.declare_dram_parameter("k", [NB, TK, D], f32, isOutput=False)
    kb_h = nc.declare_dram_parameter("kb", [NB, TK, D], bf16, isOutput=False)
    WwT_h = nc.declare_dram_parameter("WwT", [D, D], f32, isOutput=False)
    UwTb_h = nc.declare_dram_parameter("UwTb", [D, D], bf16, isOutput=False)
    Wb_h = nc.declare_dram_parameter("Wb", [D, 1], f32, isOutput=False)
    Ub_h = nc.declare_dram_parameter("Ub", [D, 1], f32, isOutput=False)
    V_h = nc.declare_dram_parameter("V", [D, 1], f32, isOutput=False)
    out_h = nc.declare_dram_parameter("out", [NB, TQ, D], f32, isOutput=True)

    with tile.TileContext(nc) as tc:
        with tc.tile_pool(name="consts", bufs=1) as consts:
            qT_sb = consts.tile([D, NB * TQ], f32)
            WwT_sb = consts.tile([D, D], f32)
            UwT_sb = consts.tile([D, D], bf16)
            Wb_sb = consts.tile([D, 1], f32)
            Ub_sb = consts.tile([D, 1], f32)
            V_sb = consts.tile([D, 1], f32)
            biasq_sb = consts.tile([D, NB * TQ], f32)
            x_sb = consts.tile([TQ, NB, D], f32)

            nc.sync.dma_start(qT_sb[:], qT_h[:])
            nc.sync.dma_start(WwT_sb[:], WwT_h[:])
            nc.sync.dma_start(UwT_sb[:], UwTb_h[:])
            nc.sync.dma_start(Wb_sb[:], Wb_h[:])
            nc.sync.dma_start(Ub_sb[:], Ub_h[:])
            nc.sync.dma_start(V_sb[:], V_h[:])

            # biasq = Ww @ q.T + Wb + Ub  -> [d, (n,q)]
            with tc.tile_pool(name="pwq", bufs=1, space="PSUM") as pwq:
                wq_ps = pwq.tile([D, NB * TQ], f32)
                nc.tensor.matmul(wq_ps[:], WwT_sb[:], qT_sb[:], start=True, stop=True)
                nc.vector.tensor_scalar(
                    biasq_sb[:],
                    wq_ps[:],
                    Wb_sb[:],
                    Ub_sb[:],
                    mybir.AluOpType.add,
                    mybir.AluOpType.add,
                )

            with (
                tc.tile_pool(name="kt", bufs=2) as ktp,
                tc.tile_pool(name="knat", bufs=2) as knp,
                tc.tile_pool(name="h", bufs=3) as hp,
                tc.tile_pool(name="soft", bufs=2) as sp,
                tc.tile_pool(name="puk", bufs=1, space="PSUM") as puk,
                tc.tile_pool(name="ps", bufs=2, space="PSUM") as ps,
                tc.tile_pool(name="px", bufs=2, space="PSUM") as px,
            ):
                for n in range(NB):
                    kT_sb = ktp.tile([D, TK], bf16, tag="kt")
                    nc.sync.dma_start(kT_sb[:], kb_h[n], transpose=True)
                    knat_sb = knp.tile([128, KC, D], f32, tag="knat")
                    nc.sync.dma_start(
                        knat_sb[:], k_h[n].rearrange("(kc p) d -> p kc d", p=128)
                    )

                    uk_ps = puk.tile([D, TK], f32, tag="uk")
                    for c in range(4):
                        nc.tensor.matmul(
                            uk_ps[:, c * 512 : (c + 1) * 512],
                            UwT_sb[:],
                            kT_sb[:, c * 512 : (c + 1) * 512],
                            start=True,
                            stop=True,
                        )

                    s_ps = ps.tile([128, TQ * KC], f32, tag="s")
                    for q in range(TQ):
                        hT = hp.tile([D, TK], f32, tag="h")
                        nc.scalar.activation(
                            hT[:],
                            uk_ps[:],
                            AF.Tanh,
                            bias=biasq_sb[:, n * TQ + q : n * TQ + q + 1],
                        )
                        for kc in range(KC):
                            nc.tensor.matmul(
                                s_ps[:, kc * TQ + q : kc * TQ + q + 1],
                                hT[:, kc * 128 : (kc + 1) * 128],
                                V_sb[:],
                                start=True,
                                stop=True,
                            )

                    exp_sb = sp.tile([128, KC, TQ], f32, tag="exp")
                    nc.scalar.activation(
                        exp_sb[:].rearrange("p kc q -> p (kc q)"), s_ps[:], AF.Exp
                    )
                    z_sb = sp.tile([128, KC], f32, tag="z")
                    nc.vector.tensor_reduce(
                        z_sb[:], exp_sb[:], mybir.AxisListType.X, mybir.AluOpType.add
                    )
                    r_sb = sp.tile([128, KC], f32, tag="r")
                    nc.vector.reciprocal(r_sb[:], z_sb[:])
                    wT_sb = sp.tile([128, KC, TQ], f32, tag="w")
                    nc.vector.tensor_tensor(
                        wT_sb[:],
                        exp_sb[:],
                        r_sb[:, :, None].to_broadcast((128, KC, TQ)),
                        mybir.AluOpType.mult,
                    )

                    x_ps = px.tile([TQ, D], f32, tag="x")
                    for kc in range(KC):
                        nc.tensor.matmul(
                            x_ps[:],
                            wT_sb[:, kc, :],
                            knat_sb[:, kc, :],
                            start=(kc == 0),
                            stop=(kc == KC - 1),
                        )
                    nc.vector.tensor_copy(x_sb[:, n, :], x_ps[:])

            nc.sync.dma_start(out_h.rearrange("n q d -> q n d"), x_sb[:])

    nc.compile()
    return nc


def _get_runner():
    """Build the Bass program once and return a persistent callable
    in_maps -> list[dict] running on 8 cores via PJRT (axon)."""
    if "runner" in _CACHE:
        return _CACHE["runner"]

    import jax
    from jax.sharding import Mesh, PartitionSpec
    from jax.experimental.shard_map import shard_map
    from concourse import mybir, bass2jax
    from concourse.bass2jax import _bass_exec_p, install_neuronx_cc_hook, partition_id_tensor

    nc = _build_nc()
    install_neuronx_cc_hook()

    partition_name = nc.partition_id_tensor.name if nc.partition_id_tensor else None
    in_names, out_names, out_avals, zero_outs = [], [], [], []
    for alloc in nc.m.functions[0].allocations:
        if not isinstance(alloc, mybir.MemoryLocationSet):
            continue
        name = alloc.memorylocations[0].name
        if alloc.kind == "ExternalInput":
            if name != partition_name:
                in_names.append(name)
        elif alloc.kind == "ExternalOutput":
            shape = tuple(alloc.tensor_shape)
            dtype = mybir.dt.np(alloc.dtype)
            out_names.append(name)
            out_avals.append(jax.core.ShapedArray(shape, dtype))
            zero_outs.append(np.zeros(shape, dtype))
    n_params = len(in_names)
    n_outs = len(out_avals)
    in_names_all = list(in_names) + out_names
    if partition_name is not None:
        in_names_all.append(partition_name)

    def _body(*args):
        operands = list(args)
        if partition_name is not None:
            operands.append(partition_id_tensor())
        outs = _bass_exec_p.bind(
            *operands,
            out_avals=tuple(out_avals),
            in_names=tuple(in_names_all),
            out_names=tuple(out_names),
            lowering_input_output_aliases=(),
            sim_require_finite=True,
            sim_require_nnan=True,
            nc=nc,
        )
        return tuple(outs)

    devices = jax.devices()[:N_CORES]
    assert len(devices) == N_CORES
    mesh = Mesh(np.asarray(devices), ("core",))
    donate = tuple(range(n_params, n_params + n_outs))
    sharded = jax.jit(
        shard_map(
            _body,
            mesh=mesh,
            in_specs=(PartitionSpec("core"),) * (n_params + n_outs),
            out_specs=(PartitionSpec("core"),) * n_outs,
            check_rep=False,
        ),
        donate_argnums=donate,
        keep_unused=True,
    )

    def run(in_maps, async_out=False):
        per_core = [[np.asarray(m[name]) for name in in_names] for m in in_maps]
        concat_in = [
            np.concatenate([per_core[c][i] for c in range(N_CORES)], axis=0)
            for i in range(n_params)
        ]
        concat_zeros = [
            np.zeros((N_CORES * z.shape[0], *z.shape[1:]), z.dtype) for z in zero_outs
        ]
        out_arrs = sharded(*concat_in, *concat_zeros)
        if async_out:
            return out_arrs
        return [
            {
                name: np.asarray(out_arrs[i]).reshape(N_CORES, *out_avals[i].shape)[c]
                for i, name in enumerate(out_names)
            }
            for c in range(N_CORES)
        ]

    _CACHE["runner"] = run
    return run


def _shard_inputs(q, k, Ww, Wb, Uw, Ub, Vw):
    WwT = np.ascontiguousarray(Ww.T)
    UwTb = np.ascontiguousarray(Uw.T).astype(ml_dtypes.bfloat16)
    Wbc = np.ascontiguousarray(Wb.reshape(D, 1))
    Ubc = np.ascontiguousarray(Ub.reshape(D, 1))
    Vc = np.ascontiguousarray(Vw.reshape(D)[:, None])
    in_maps = []
    for i in range(N_CORES):
        qc = q[NB * i : NB * (i + 1)]
        kc = np.ascontiguousarray(k[NB * i : NB * (i + 1)])
        in_maps.append(
            {
                "qT": np.ascontiguousarray(qc.reshape(NB * TQ, D).T),
                "k": kc,
                "kb": kc.astype(ml_dtypes.bfloat16),
                "WwT": WwT,
                "UwTb": UwTb,
                "Wb": Wbc,
                "Ub": Ubc,
                "V": Vc,
            }
        )
    return in_maps


def kernel(q, k, mask, Ww, Wb, Uw, Ub, Vw, Vb):
    q = np.asarray(q, np.float32)
    k = np.asarray(k, np.float32)
    # mask and Vb shift scores uniformly across the softmax (q) axis,
    # so they cancel exactly; they are not used on device.
    run = _get_runner()
    in_maps = _shard_inputs(
        q,
        k,
        np.asarray(Ww, np.float32),
        np.asarray(Wb, np.float32),
        np.asarray(Uw, np.float32),
        np.asarray(Ub, np.float32),
        np.asarray(Vw, np.float32),
    )
    res = run(in_maps)
    return np.concatenate([res[i]["out"] for i in range(N_CORES)], axis=0)


# revision 5
# speedup vs baseline: 9.8887x; 9.8887x over previous
"""Bahdanau additive cross-attention pooler on 8 TRN2 NeuronCores.

Math (per batch n):
    wq = q @ Ww.T + Wb                      [16, 128]
    uk = k @ Uw.T + Ub                      [2048, 128]
    s[q,k] = V . tanh(wq[q] + uk[k])        [16, 2048]
    w = softmax(s, axis=q)                  (mask and Vb terms are constant in
                                             q, so they cancel in the softmax
                                             exactly -> never computed)
    x[q] = sum_k w[q,k] * k[k]              [16, 128]

Distribution: batch N=32 data-parallel over 8 cores (4 batches/core),
params replicated. No collectives.

Per-core device pipeline (all layouts transposed so d lives on partitions):
    biasq[d, (n,q)] = WwT.T @ qT + Wb + Ub      (one matmul + tensor_scalar)
    per n:
      kT[d, k]   <- DMA-xbar-transpose of bf16 copy of k
      knat[p,kc,d] <- natural k (fp32)
      uk_psum[d, k] = UwT.T @ kT                 (PSUM, 4 matmuls)
      per q: hT = tanh(uk_psum + biasq[:, nq])   (one fused ACT instr, PSUM src)
             s_psum[:, kc*16+q] = hT_chunk.T @ V (16 small matmuls)
      exp_sb = exp(s_psum); Z = rowsum by kc-group; wT = exp_sb * recip(Z)
      x_psum[q, d] = sum_kc wT_chunk.T @ knat_chunk
"""

import sys

sys.path.insert(0, "/opt/trn_rl_repo")

import numpy as np
import ml_dtypes

N_CORES = 8
NB = 4          # batches per core
TQ = 16
TK = 2048
D = 128
KC = TK // 128  # 16 k-chunks of 128

_CACHE = {}


def _build_nc():
    import concourse.bacc as bacc
    import concourse.tile as tile
    from concourse import mybir

    f32 = mybir.dt.float32
    bf16 = mybir.dt.bfloat16
    AF = mybir.ActivationFunctionType

    nc = bacc.Bacc()
    qT_h = nc.declare_dram_parameter("qT", [D, NB * TQ], f32, isOutput=False)
    k_h = nc.declare_dram_parameter("k", [NB, TK, D], f32, isOutput=False)
    kb_h = nc.declare_dram_parameter("kb", [NB, TK, D], bf16, isOutput=False)
    WwT_h = nc.declare_dram_parameter("WwT", [D, D], f32, isOutput=False)
    UwTb_h = nc.declare_dram_parameter("UwTb", [D, D], bf16, isOutput=False)
    Wb_h = nc.declare_dram_parameter("Wb", [D, 1], f32, isOutput=False)
    Ub_h = nc.declare_dram_parameter("Ub", [D, 1], f32, isOutput=False)
    V_h = nc.declare_dram_parameter("V", [D, 1], f32, isOutput=False)
    out_h = nc.declare_dram_parameter("out", [NB, TQ, D], f32, isOutput=True)

    with tile.TileContext(nc) as tc:
        with tc.tile_pool(name="consts", bufs=1) as consts:
            qT_sb = consts.tile([D, NB * TQ], f32)
            WwT_sb = consts.tile([D, D], f32)
            UwT_sb = consts.tile([D, D], bf16)
            Wb_sb = consts.tile([D, 1], f32)
            Ub_sb = consts.tile([D, 1], f32)
            V_sb = consts.tile([D, 1], f32)
            biasq_sb = consts.tile([D, NB * TQ], f32)
            x_sb = consts.tile([TQ, NB, D], f32)

            nc.sync.dma_start(qT_sb[:], qT_h[:])
            nc.sync.dma_start(WwT_sb[:], WwT_h[:])
            nc.sync.dma_start(UwT_sb[:], UwTb_h[:])
            nc.sync.dma_start(Wb_sb[:], Wb_h[:])
            nc.sync.dma_start(Ub_sb[:], Ub_h[:])
            nc.sync.dma_start(V_sb[:], V_h[:])

            # biasq = Ww @ q.T + Wb + Ub  -> [d, (n,q)]
            with tc.tile_pool(name="pwq", bufs=1, space="PSUM") as pwq:
                wq_ps = pwq.tile([D, NB * TQ], f32)
                nc.tensor.matmul(wq_ps[:], WwT_sb[:], qT_sb[:], start=True, stop=True)
                nc.vector.tensor_scalar(
                    biasq_sb[:],
                    wq_ps[:],
                    Wb_sb[:],
                    Ub_sb[:],
                    mybir.AluOpType.add,
                    mybir.AluOpType.add,
                )

            with (
                tc.tile_pool(name="kt", bufs=2) as ktp,
                tc.tile_pool(name="knat", bufs=2) as knp,
                tc.tile_pool(name="h", bufs=3) as hp,
                tc.tile_pool(name="soft", bufs=2) as sp,
                tc.tile_pool(name="puk", bufs=1, space="PSUM") as puk,
                tc.tile_pool(name="ps", bufs=2, space="PSUM") as ps,
                tc.tile_pool(name="px", bufs=2, space="PSUM") as px,
            ):
                for n in range(NB):
                    kT_sb = ktp.tile([D, TK], bf16, tag="kt")
                    nc.sync.dma_start(kT_sb[:], kb_h[n], transpose=True)
                    knat_sb = knp.tile([128, KC, D], f32, tag="knat")
                    nc.sync.dma_start(
                        knat_sb[:], k_h[n].rearrange("(kc p) d -> p kc d", p=128)
                    )

                    uk_ps = puk.tile([D, TK], f32, tag="uk")
                    for c in range(4):
                        nc.tensor.matmul(
                            uk_ps[:, c * 512 : (c + 1) * 512],
                            UwT_sb[:],
                            kT_sb[:, c * 512 : (c + 1) * 512],
                            start=True,
                            stop=True,
                        )

                    s_ps = ps.tile([128, TQ * KC], f32, tag="s")
                    for q in range(TQ):
                        hT = hp.tile([D, TK], f32, tag="h")
                        nc.scalar.activation(
                            hT[:],
                            uk_ps[:],
                            AF.Tanh,
                            bias=biasq_sb[:, n * TQ + q : n * TQ + q + 1],
                        )
                        for kc in range(KC):
                            nc.tensor.matmul(
                                s_ps[:, kc * TQ + q : kc * TQ + q + 1],
                                hT[:, kc * 128 : (kc + 1) * 128],
                                V_sb[:],
                                start=True,
                                stop=True,
                            )

                    exp_sb = sp.tile([128, KC, TQ], f32, tag="exp")
                    nc.scalar.activation(
                        exp_sb[:].rearrange("p kc q -> p (kc q)"), s_ps[:], AF.Exp
                    )
                    z_sb = sp.tile([128, KC], f32, tag="z")
                    nc.vector.tensor_reduce(
                        z_sb[:], exp_sb[:], mybir.AxisListType.X, mybir.AluOpType.add
                    )
                    r_sb = sp.tile([128, KC], f32, tag="r")
                    nc.vector.reciprocal(r_sb[:], z_sb[:])
                    wT_sb = sp.tile([128, KC, TQ], f32, tag="w")
                    nc.vector.tensor_tensor(
                        wT_sb[:],
                        exp_sb[:],
                        r_sb[:, :, None].to_broadcast((128, KC, TQ)),
                        mybir.AluOpType.mult,
                    )

                    x_ps = px.tile([TQ, D], f32, tag="x")
                    for kc in range(KC):
                        nc.tensor.matmul(
                            x_ps[:],
                            wT_sb[:, kc, :],
                            knat_sb[:, kc, :],
                            start=(kc == 0),
                            stop=(kc == KC - 1),
                        )
                    nc.vector.tensor_copy(x_sb[:, n, :], x_ps[:])

            nc.sync.dma_start(out_h.rearrange("n q d -> q n d"), x_sb[:])

    nc.compile()
    return nc


def _get_runner():
    """Build the Bass program once and return a persistent callable
    in_maps -> list[dict] running on 8 cores via PJRT (axon)."""
    if "runner" in _CACHE:
        return _CACHE["runner"]

    import jax
    import jax.numpy as jnp
    from jax.sharding import Mesh, PartitionSpec, NamedSharding
    from jax.experimental.shard_map import shard_map
    from concourse import mybir
    from concourse.bass2jax import _bass_exec_p, install_neuronx_cc_hook, partition_id_tensor

    nc = _build_nc()
    install_neuronx_cc_hook()

    partition_name = nc.partition_id_tensor.name if nc.partition_id_tensor else None
    in_names, out_names, out_avals = [], [], []
    for alloc in nc.m.functions[0].allocations:
        if not isinstance(alloc, mybir.MemoryLocationSet):
            continue
        name = alloc.memorylocations[0].name
        if alloc.kind == "ExternalInput":
            if name != partition_name:
                in_names.append(name)
        elif alloc.kind == "ExternalOutput":
            shape = tuple(alloc.tensor_shape)
            dtype = mybir.dt.np(alloc.dtype)
            out_names.append(name)
            out_avals.append(jax.core.ShapedArray(shape, dtype))
    n_params = len(in_names)
    in_names_all = list(in_names) + out_names
    if partition_name is not None:
        in_names_all.append(partition_name)

    def _body(*args):
        operands = list(args)
        if partition_name is not None:
            operands.append(partition_id_tensor())
        outs = _bass_exec_p.bind(
            *operands,
            out_avals=tuple(out_avals),
            in_names=tuple(in_names_all),
            out_names=tuple(out_names),
            lowering_input_output_aliases=(),
            sim_require_finite=True,
            sim_require_nnan=True,
            nc=nc,
        )
        return tuple(outs)

    devices = jax.devices()[:N_CORES]
    assert len(devices) == N_CORES
    mesh = Mesh(np.asarray(devices), ("core",))
    sharding = NamedSharding(mesh, PartitionSpec("core"))
    n_outs = len(out_names)
    sharded = jax.jit(
        shard_map(
            _body,
            mesh=mesh,
            in_specs=(PartitionSpec("core"),) * (n_params + n_outs),
            out_specs=(PartitionSpec("core"),) * n_outs,
            check_rep=False,
        ),
        keep_unused=True,
    )

    def prepare(in_maps):
        """Concatenate per-core inputs (plus output zero-buffers) and move
        them to device once; safe to reuse across calls (no donation)."""
        per_core = [[np.asarray(m[name]) for name in in_names] for m in in_maps]
        concat_in = [
            np.concatenate([per_core[c][i] for c in range(N_CORES)], axis=0)
            for i in range(n_params)
        ]
        concat_in += [
            np.zeros((N_CORES * a.shape[0], *a.shape[1:]), a.dtype) for a in out_avals
        ]
        return [jax.device_put(a, sharding) for a in concat_in]

    def run_prepared(dev_args, async_out=False):
        out_arrs = sharded(*dev_args)
        if async_out:
            return out_arrs
        return [
            {
                name: np.asarray(out_arrs[i]).reshape(N_CORES, *out_avals[i].shape)[c]
                for i, name in enumerate(out_names)
            }
            for c in range(N_CORES)
        ]

    def run(in_maps, async_out=False):
        return run_prepared(prepare(in_maps), async_out=async_out)

    run.prepare = prepare
    run.run_prepared = run_prepared
    _CACHE["runner"] = run
    return run


def _shard_inputs(q, k, Ww, Wb, Uw, Ub, Vw):
    WwT = np.ascontiguousarray(Ww.T)
    UwTb = np.ascontiguousarray(Uw.T).astype(ml_dtypes.bfloat16)
    Wbc = np.ascontiguousarray(Wb.reshape(D, 1))
    Ubc = np.ascontiguousarray(Ub.reshape(D, 1))
    Vc = np.ascontiguousarray(Vw.reshape(D)[:, None])
    in_maps = []
    for i in range(N_CORES):
        qc = q[NB * i : NB * (i + 1)]
        kc = np.ascontiguousarray(k[NB * i : NB * (i + 1)])
        in_maps.append(
            {
                "qT": np.ascontiguousarray(qc.reshape(NB * TQ, D).T),
                "k": kc,
                "kb": kc.astype(ml_dtypes.bfloat16),
                "WwT": WwT,
                "UwTb": UwTb,
                "Wb": Wbc,
                "Ub": Ubc,
                "V": Vc,
            }
        )
    return in_maps


def kernel(q, k, mask, Ww, Wb, Uw, Ub, Vw, Vb):
    q = np.asarray(q, np.float32)
    k = np.asarray(k, np.float32)
    # mask and Vb shift scores uniformly across the softmax (q) axis,
    # so they cancel exactly; they are not used on device.
    run = _get_runner()
    in_maps = _shard_inputs(
        q,
        k,
        np.asarray(Ww, np.float32),
        np.asarray(Wb, np.float32),
        np.asarray(Uw, np.float32),
        np.asarray(Ub, np.float32),
        np.asarray(Vw, np.float32),
    )
    res = run(in_maps)
    return np.concatenate([res[i]["out"] for i in range(N_CORES)], axis=0)


# revision 12
# speedup vs baseline: 2144.6843x; 216.8833x over previous
"""Bahdanau additive cross-attention pooler on 8 TRN2 NeuronCores.

Math (per batch n):
    wq = q @ Ww.T + Wb                      [16, 128]
    uk = k @ Uw.T + Ub                      [2048, 128]
    s[q,k] = V . tanh(wq[q] + uk[k])        [16, 2048]
    w = softmax(s, axis=q)                  (mask and Vb terms are constant in
                                             q, so they cancel in the softmax
                                             exactly -> never computed)
    x[q] = sum_k w[q,k] * k[k]              [16, 128]

Distribution: batch N=32 data-parallel over 8 cores (4 batches/core),
params replicated. No collectives.

Per-core device pipeline (all layouts transposed so d lives on partitions):
    biasq[d, (n,q)] = WwT.T @ qT + Wb + Ub      (one matmul + tensor_scalar)
    per n:
      kT[d, k]   <- DMA-xbar-transpose of bf16 copy of k
      knat[p,kc,d] <- natural k (fp32)
      uk_psum[d, k] = UwT.T @ kT                 (PSUM, 4 matmuls)
      per q: hT = tanh(uk_psum + biasq[:, nq])   (one fused ACT instr, PSUM src)
             s_psum[:, kc*16+q] = hT_chunk.T @ V (16 small matmuls)
      exp_sb = exp(s_psum); Z = rowsum by kc-group; wT = exp_sb * recip(Z)
      x_psum[q, d] = sum_kc wT_chunk.T @ knat_chunk
"""

import sys

sys.path.insert(0, "/opt/trn_rl_repo")

import numpy as np
import ml_dtypes

N_CORES = 8
NB = 4          # batches per core
TQ = 16
TK = 2048
D = 128
KC = TK // 128  # 16 k-chunks of 128

_CACHE = {}


def _build_nc(loop_iters=None):
    from contextlib import ExitStack

    import concourse.bacc as bacc
    import concourse.tile as tile
    from concourse import mybir

    f32 = mybir.dt.float32
    bf16 = mybir.dt.bfloat16
    AF = mybir.ActivationFunctionType

    nc = bacc.Bacc()
    qT_h = nc.declare_dram_parameter("qT", [D, NB * TQ], f32, isOutput=False)
    k_h = nc.declare_dram_parameter("k", [NB, TK, D], f32, isOutput=False)
    kb_h = nc.declare_dram_parameter("kb", [NB, TK, D], bf16, isOutput=False)
    WwT_h = nc.declare_dram_parameter("WwT", [D, D], f32, isOutput=False)
    UwTb_h = nc.declare_dram_parameter("UwTb", [D, D], bf16, isOutput=False)
    Wb_h = nc.declare_dram_parameter("Wb", [D, 1], f32, isOutput=False)
    Ub_h = nc.declare_dram_parameter("Ub", [D, 1], f32, isOutput=False)
    V_h = nc.declare_dram_parameter("V", [D, 1], f32, isOutput=False)
    out_h = nc.declare_dram_parameter("out", [NB, TQ, D], f32, isOutput=True)

    with tile.TileContext(nc) as tc:
        with tc.tile_pool(name="consts", bufs=1) as consts:
            qT_sb = consts.tile([D, NB * TQ], f32)
            WwT_sb = consts.tile([D, D], f32)
            UwT_sb = consts.tile([D, D], bf16)
            Wb_sb = consts.tile([D, 1], f32)
            Ub_sb = consts.tile([D, 1], f32)
            V_sb = consts.tile([D, 1], f32)
            biasq_sb = consts.tile([D, NB * TQ], f32)
            x_sb = consts.tile([TQ, NB, D], f32)

            nc.sync.dma_start(qT_sb[:], qT_h[:])
            nc.sync.dma_start(WwT_sb[:], WwT_h[:])
            nc.sync.dma_start(UwT_sb[:], UwTb_h[:])
            nc.sync.dma_start(Wb_sb[:], Wb_h[:])
            nc.sync.dma_start(Ub_sb[:], Ub_h[:])
            nc.sync.dma_start(V_sb[:], V_h[:])

            # biasq = Ww @ q.T + Wb + Ub  -> [d, (n,q)]
            with tc.tile_pool(name="pwq", bufs=1, space="PSUM") as pwq:
                wq_ps = pwq.tile([D, NB * TQ], f32)
                nc.tensor.matmul(wq_ps[:], WwT_sb[:], qT_sb[:], start=True, stop=True)
                nc.vector.tensor_scalar(
                    biasq_sb[:],
                    wq_ps[:],
                    Wb_sb[:],
                    Ub_sb[:],
                    mybir.AluOpType.add,
                    mybir.AluOpType.add,
                )

            with (
                tc.tile_pool(name="kt", bufs=2) as ktp,
                tc.tile_pool(name="knat", bufs=2) as knp,
                tc.tile_pool(name="h", bufs=3) as hp,
                tc.tile_pool(name="soft", bufs=2) as sp,
                tc.tile_pool(name="puk", bufs=1, space="PSUM") as puk,
                tc.tile_pool(name="ps", bufs=2, space="PSUM") as ps,
                tc.tile_pool(name="px", bufs=2, space="PSUM") as px,
                ExitStack() as loop_ctx,
            ):
                if loop_iters is not None:
                    loop_ctx.enter_context(
                        tc.For_i(
                            0,
                            loop_iters,
                            1,
                            hint_engines=(
                                mybir.EngineType.PE,
                                mybir.EngineType.Activation,
                                mybir.EngineType.DVE,
                                mybir.EngineType.SP,
                            ),
                        )
                    )
                for n in range(NB):
                    kT_sb = ktp.tile([D, TK], bf16, tag="kt")
                    nc.sync.dma_start(kT_sb[:], kb_h[n], transpose=True)
                    knat_sb = knp.tile([128, KC, D], f32, tag="knat")
                    nc.sync.dma_start(
                        knat_sb[:], k_h[n].rearrange("(kc p) d -> p kc d", p=128)
                    )

                    uk_ps = puk.tile([D, TK], f32, tag="uk")
                    for c in range(4):
                        nc.tensor.matmul(
                            uk_ps[:, c * 512 : (c + 1) * 512],
                            UwT_sb[:],
                            kT_sb[:, c * 512 : (c + 1) * 512],
                            start=True,
                            stop=True,
                        )

                    s_ps = ps.tile([128, TQ * KC], f32, tag="s")
                    for q in range(TQ):
                        hT = hp.tile([D, TK], f32, tag="h")
                        nc.scalar.activation(
                            hT[:],
                            uk_ps[:],
                            AF.Tanh,
                            bias=biasq_sb[:, n * TQ + q : n * TQ + q + 1],
                        )
                        for kc in range(KC):
                            nc.tensor.matmul(
                                s_ps[:, kc * TQ + q : kc * TQ + q + 1],
                                hT[:, kc * 128 : (kc + 1) * 128],
                                V_sb[:],
                                start=True,
                                stop=True,
                            )

                    exp_sb = sp.tile([128, KC, TQ], f32, tag="exp")
                    nc.scalar.activation(
                        exp_sb[:].rearrange("p kc q -> p (kc q)"), s_ps[:], AF.Exp
                    )
                    z_sb = sp.tile([128, KC], f32, tag="z")
                    nc.vector.tensor_reduce(
                        z_sb[:], exp_sb[:], mybir.AxisListType.X, mybir.AluOpType.add
                    )
                    r_sb = sp.tile([128, KC], f32, tag="r")
                    nc.vector.reciprocal(r_sb[:], z_sb[:])
                    wT_sb = sp.tile([128, KC, TQ], f32, tag="w")
                    nc.vector.tensor_tensor(
                        wT_sb[:],
                        exp_sb[:],
                        r_sb[:, :, None].to_broadcast((128, KC, TQ)),
                        mybir.AluOpType.mult,
                    )

                    x_ps = px.tile([TQ, D], f32, tag="x")
                    for kc in range(KC):
                        nc.tensor.matmul(
                            x_ps[:],
                            wT_sb[:, kc, :],
                            knat_sb[:, kc, :],
                            start=(kc == 0),
                            stop=(kc == KC - 1),
                        )
                    nc.vector.tensor_copy(x_sb[:, n, :], x_ps[:])

                nc.sync.dma_start(out_h.rearrange("n q d -> q n d"), x_sb[:])

    nc.compile()
    return nc


def _get_runner(loop_iters=None):
    """Build the Bass program once and return a persistent callable
    in_maps -> list[dict] running on 8 cores via PJRT (axon)."""
    key = ("runner", loop_iters)
    if key in _CACHE:
        return _CACHE[key]

    import jax
    import jax.numpy as jnp
    from jax.sharding import Mesh, PartitionSpec, NamedSharding
    from jax.experimental.shard_map import shard_map
    from concourse import mybir
    from concourse.bass2jax import _bass_exec_p, install_neuronx_cc_hook, partition_id_tensor

    nc = _build_nc(loop_iters)
    install_neuronx_cc_hook()

    partition_name = nc.partition_id_tensor.name if nc.partition_id_tensor else None
    in_names, out_names, out_avals = [], [], []
    for alloc in nc.m.functions[0].allocations:
        if not isinstance(alloc, mybir.MemoryLocationSet):
            continue
        name = alloc.memorylocations[0].name
        if alloc.kind == "ExternalInput":
            if name != partition_name:
                in_names.append(name)
        elif alloc.kind == "ExternalOutput":
            shape = tuple(alloc.tensor_shape)
            dtype = mybir.dt.np(alloc.dtype)
            out_names.append(name)
            out_avals.append(jax.core.ShapedArray(shape, dtype))
    n_params = len(in_names)
    in_names_all = list(in_names) + out_names
    if partition_name is not None:
        in_names_all.append(partition_name)

    def _body(*args):
        operands = list(args)
        if partition_name is not None:
            operands.append(partition_id_tensor())
        outs = _bass_exec_p.bind(
            *operands,
            out_avals=tuple(out_avals),
            in_names=tuple(in_names_all),
            out_names=tuple(out_names),
            lowering_input_output_aliases=(),
            sim_require_finite=True,
            sim_require_nnan=True,
            nc=nc,
        )
        return tuple(outs)

    devices = jax.devices()[:N_CORES]
    assert len(devices) == N_CORES
    mesh = Mesh(np.asarray(devices), ("core",))
    sharding = NamedSharding(mesh, PartitionSpec("core"))
    n_outs = len(out_names)
    sharded = jax.jit(
        shard_map(
            _body,
            mesh=mesh,
            in_specs=(PartitionSpec("core"),) * (n_params + n_outs),
            out_specs=(PartitionSpec("core"),) * n_outs,
            check_rep=False,
        ),
        keep_unused=True,
    )

    def prepare(in_maps):
        """Concatenate per-core inputs (plus output zero-buffers) and move
        them to device once; safe to reuse across calls (no donation)."""
        per_core = [[np.asarray(m[name]) for name in in_names] for m in in_maps]
        concat_in = [
            np.concatenate([per_core[c][i] for c in range(N_CORES)], axis=0)
            for i in range(n_params)
        ]
        concat_in += [
            np.zeros((N_CORES * a.shape[0], *a.shape[1:]), a.dtype) for a in out_avals
        ]
        return [jax.device_put(a, sharding) for a in concat_in]

    def run_prepared(dev_args, async_out=False):
        out_arrs = sharded(*dev_args)
        if async_out:
            return out_arrs
        return [
            {
                name: np.asarray(out_arrs[i]).reshape(N_CORES, *out_avals[i].shape)[c]
                for i, name in enumerate(out_names)
            }
            for c in range(N_CORES)
        ]

    def run(in_maps, async_out=False):
        return run_prepared(prepare(in_maps), async_out=async_out)

    run.prepare = prepare
    run.run_prepared = run_prepared
    _CACHE[key] = run
    return run


def _shard_inputs(q, k, Ww, Wb, Uw, Ub, Vw):
    WwT = np.ascontiguousarray(Ww.T)
    UwTb = np.ascontiguousarray(Uw.T).astype(ml_dtypes.bfloat16)
    Wbc = np.ascontiguousarray(Wb.reshape(D, 1))
    Ubc = np.ascontiguousarray(Ub.reshape(D, 1))
    Vc = np.ascontiguousarray(Vw.reshape(D)[:, None])
    in_maps = []
    for i in range(N_CORES):
        qc = q[NB * i : NB * (i + 1)]
        kc = np.ascontiguousarray(k[NB * i : NB * (i + 1)])
        in_maps.append(
            {
                "qT": np.ascontiguousarray(qc.reshape(NB * TQ, D).T),
                "k": kc,
                "kb": kc.astype(ml_dtypes.bfloat16),
                "WwT": WwT,
                "UwTb": UwTb,
                "Wb": Wbc,
                "Ub": Ubc,
                "V": Vc,
            }
        )
    return in_maps


def kernel(q, k, mask, Ww, Wb, Uw, Ub, Vw, Vb):
    q = np.asarray(q, np.float32)
    k = np.asarray(k, np.float32)
    # mask and Vb shift scores uniformly across the softmax (q) axis,
    # so they cancel exactly; they are not used on device.
    run = _get_runner()
    in_maps = _shard_inputs(
        q,
        k,
        np.asarray(Ww, np.float32),
        np.asarray(Wb, np.float32),
        np.asarray(Uw, np.float32),
        np.asarray(Ub, np.float32),
        np.asarray(Vw, np.float32),
    )
    res = run(in_maps)
    return np.concatenate([res[i]["out"] for i in range(N_CORES)], axis=0)


# revision 28
# speedup vs baseline: 3780.1216x; 1.7626x over previous
"""Bahdanau additive cross-attention pooler on 8 TRN2 NeuronCores.

Math (per batch n):
    wq = q @ Ww.T + Wb                      [16, 128]
    uk = k @ Uw.T + Ub                      [2048, 128]
    s[q,k] = V . tanh(wq[q] + uk[k])        [16, 2048]
    w = softmax(s, axis=q)                  (mask and Vb terms are constant in
                                             q, so they cancel in the softmax
                                             exactly -> never computed)
    x[q] = sum_k w[q,k] * k[k]              [16, 128]

Distribution: batch N=32 data-parallel over 8 cores (4 batches/core),
params replicated. No collectives.

Per-core device pipeline (all layouts transposed so d lives on partitions):
    biasq[d, (n,q)] = WwT.T @ qT + Wb + Ub      (one matmul + tensor_scalar)
    per n:
      kT[d, k]   <- DMA-xbar-transpose of bf16 copy of k
      knat[p,kc,d] <- natural k (fp32)
      uk_psum[d, k] = UwT.T @ kT                 (PSUM, 4 matmuls)
      per q: hT = tanh(uk_psum + biasq[:, nq])   (one fused ACT instr, PSUM src)
             s_psum[:, kc*16+q] = hT_chunk.T @ V (16 small matmuls)
      exp_sb = exp(s_psum); Z = rowsum by kc-group; wT = exp_sb * recip(Z)
      x_psum[q, d] = sum_kc wT_chunk.T @ knat_chunk
"""

import sys

sys.path.insert(0, "/opt/trn_rl_repo")

import numpy as np
import ml_dtypes

N_CORES = 8
NB = 4          # batches per core
TQ = 16
TK = 2048
D = 128
KC = TK // 128  # 16 k-chunks of 128

_CACHE = {}


def _build_nc(loop_iters=None, trace_sim=False, ablate=()):
    from contextlib import ExitStack

    import concourse.bacc as bacc
    import concourse.tile as tile
    from concourse import mybir
    from concourse.masks import make_identity

    ablate = set(ablate)

    f32 = mybir.dt.float32
    bf16 = mybir.dt.bfloat16
    AF = mybir.ActivationFunctionType

    nc = bacc.Bacc()
    qT_h = nc.declare_dram_parameter("qT", [D, NB * TQ], f32, isOutput=False)
    k_h = nc.declare_dram_parameter("k", [NB, TK, D], f32, isOutput=False)
    kb_h = nc.declare_dram_parameter("kb", [NB, TK, D], bf16, isOutput=False)
    WwT_h = nc.declare_dram_parameter("WwT", [D, D], f32, isOutput=False)
    UwTb_h = nc.declare_dram_parameter("UwTb", [D, D], bf16, isOutput=False)
    Wb_h = nc.declare_dram_parameter("Wb", [D, 1], f32, isOutput=False)
    Ub_h = nc.declare_dram_parameter("Ub", [D, 1], f32, isOutput=False)
    VQ_h = nc.declare_dram_parameter("VQ", [D, TQ * TQ], f32, isOutput=False)
    out_h = nc.declare_dram_parameter("out", [NB, TQ, D], f32, isOutput=True)

    with tile.TileContext(nc, trace_sim=trace_sim) as tc:
        with tc.tile_pool(name="consts", bufs=1) as consts:
            qT_sb = consts.tile([D, NB * TQ], f32)
            WwT_sb = consts.tile([D, D], f32)
            UwT_sb = consts.tile([D, D], bf16)
            Wb_sb = consts.tile([D, 1], f32)
            Ub_sb = consts.tile([D, 1], f32)
            VQ_sb = consts.tile([D, TQ * TQ], f32)
            biasq_sb = consts.tile([D, NB * TQ], f32)
            x_sb = consts.tile([TQ, NB, D], f32)
            ident_sb = consts.tile([TQ, TQ], f32)

            nc.sync.dma_start(qT_sb[:], qT_h[:])
            nc.sync.dma_start(WwT_sb[:], WwT_h[:])
            nc.sync.dma_start(UwT_sb[:], UwTb_h[:])
            nc.sync.dma_start(Wb_sb[:], Wb_h[:])
            nc.sync.dma_start(Ub_sb[:], Ub_h[:])
            nc.sync.dma_start(VQ_sb[:], VQ_h[:])
            make_identity(nc, ident_sb[:])

            # biasq = Ww @ q.T + Wb + Ub  -> [d, (n,q)]
            with tc.tile_pool(name="pwq", bufs=1, space="PSUM") as pwq:
                wq_ps = pwq.tile([D, NB * TQ], f32)
                nc.tensor.matmul(wq_ps[:], WwT_sb[:], qT_sb[:], start=True, stop=True)
                nc.vector.tensor_scalar(
                    biasq_sb[:],
                    wq_ps[:],
                    Wb_sb[:],
                    Ub_sb[:],
                    mybir.AluOpType.add,
                    mybir.AluOpType.add,
                )

            with (
                tc.tile_pool(name="kt", bufs=2) as ktp,
                tc.tile_pool(name="knat", bufs=2) as knp,
                tc.tile_pool(name="uk", bufs=2) as ukp,
                tc.tile_pool(name="h", bufs=3) as hp,
                tc.tile_pool(name="soft", bufs=2) as sp,
                tc.tile_pool(name="pukc", bufs=2, space="PSUM") as pukc,
                tc.tile_pool(name="ps", bufs=1, space="PSUM") as ps,
                tc.tile_pool(name="psT", bufs=1, space="PSUM") as psT,
                tc.tile_pool(name="px", bufs=1, space="PSUM") as px,
                ExitStack() as loop_ctx,
            ):
                if loop_iters is not None:
                    loop_ctx.enter_context(
                        tc.For_i(
                            0,
                            loop_iters,
                            1,
                            hint_engines=(
                                mybir.EngineType.PE,
                                mybir.EngineType.Activation,
                                mybir.EngineType.DVE,
                                mybir.EngineType.SP,
                            ),
                        )
                    )
                for n in range(NB):
                    kT_sb = ktp.tile([D, TK], bf16, tag="kt")
                    if "kT" in ablate:
                        # plain (wrong-layout) load, timing-only ablation
                        nc.sync.dma_start(
                            kT_sb[:].rearrange("p (a d) -> p a d", d=128),
                            kb_h[n].rearrange("(a p) d -> p a d", p=128),
                        )
                    else:
                        nc.sync.dma_start(kT_sb[:], kb_h[n], transpose=True)
                    knat_sb = knp.tile([128, KC, D], f32, tag="knat")
                    nc.sync.dma_start(
                        knat_sb[:], k_h[n].rearrange("(kc p) d -> p kc d", p=128)
                    )

                    # uk = Uw @ k.T, chunked through PSUM into SBUF
                    uk_sb = ukp.tile([D, TK], f32, tag="uk")
                    for c in range(4):
                        uk_ps = pukc.tile([D, 512], f32, tag="ukc")
                        nc.tensor.matmul(
                            uk_ps[:],
                            UwT_sb[:],
                            kT_sb[:, c * 512 : (c + 1) * 512],
                            start=True,
                            stop=True,
                        )
                        nc.vector.tensor_copy(uk_sb[:, c * 512 : (c + 1) * 512], uk_ps[:])

                    # scores: V stationary, h moving -> s_ps[q, k] rows
                    s_ps = ps.tile([TQ, TK], f32, tag="s")
                    for q in range(TQ):
                        hT = hp.tile([D, TK], f32, tag="h")
                        if "tanh" in ablate:
                            nc.scalar.activation(
                                hT[:, :2],
                                uk_sb[:, :2],
                                AF.Tanh,
                                bias=biasq_sb[:, n * TQ + q : n * TQ + q + 1],
                            )
                        else:
                            nc.scalar.activation(
                                hT[:],
                                uk_sb[:],
                                AF.Tanh,
                                bias=biasq_sb[:, n * TQ + q : n * TQ + q + 1],
                            )
                        if "score" in ablate:
                            if q == 0:
                                nc.tensor.matmul(
                                    s_ps[:, 0:512],
                                    VQ_sb[:, 0:TQ],
                                    hT[:, 0:512],
                                    start=True,
                                    stop=True,
                                )
                        else:
                            # one-hot V columns: writes row q, accumulates
                            for c in range(4):
                                nc.tensor.matmul(
                                    s_ps[:, c * 512 : (c + 1) * 512],
                                    VQ_sb[:, q * TQ : (q + 1) * TQ],
                                    hT[:, c * 512 : (c + 1) * 512],
                                    start=(q == 0),
                                    stop=(q == TQ - 1),
                                )

                    # transpose scores [q, k] -> [k, q] via PE
                    s_sb = sp.tile([TQ, TK], f32, tag="scopy")
                    nc.vector.tensor_copy(s_sb[:], s_ps[:])
                    sT_ps = psT.tile([128, KC, TQ], f32, tag="sT")
                    for kc in range(KC):
                        nc.tensor.transpose(
                            sT_ps[:, kc, :],
                            s_sb[:, kc * 128 : (kc + 1) * 128],
                            ident_sb[:],
                        )

                    exp_sb = sp.tile([128, KC, TQ], f32, tag="exp")
                    nc.scalar.activation(
                        exp_sb[:].rearrange("p kc q -> p (kc q)"),
                        sT_ps[:].rearrange("p kc q -> p (kc q)"),
                        AF.Exp,
                    )
                    z_sb = sp.tile([128, KC], f32, tag="z")
                    nc.vector.tensor_reduce(
                        z_sb[:], exp_sb[:], mybir.AxisListType.X, mybir.AluOpType.add
                    )
                    r_sb = sp.tile([128, KC], f32, tag="r")
                    nc.vector.reciprocal(r_sb[:], z_sb[:])
                    wT_sb = sp.tile([128, KC, TQ], f32, tag="w")
                    nc.vector.tensor_tensor(
                        wT_sb[:],
                        exp_sb[:],
                        r_sb[:, :, None].to_broadcast((128, KC, TQ)),
                        mybir.AluOpType.mult,
                    )

                    x_ps = px.tile([TQ, D], f32, tag="x")
                    for kc in range(KC):
                        nc.tensor.matmul(
                            x_ps[:],
                            wT_sb[:, kc, :],
                            knat_sb[:, kc, :],
                            start=(kc == 0),
                            stop=(kc == KC - 1),
                        )
                    nc.vector.tensor_copy(x_sb[:, n, :], x_ps[:])

                nc.sync.dma_start(out_h.rearrange("n q d -> q n d"), x_sb[:])

    nc.compile()
    return nc


def _get_runner(loop_iters=None):
    """Build the Bass program once and return a persistent callable
    in_maps -> list[dict] running on 8 cores via PJRT (axon)."""
    key = ("runner", loop_iters)
    if key in _CACHE:
        return _CACHE[key]

    import jax
    import jax.numpy as jnp
    from jax.sharding import Mesh, PartitionSpec, NamedSharding
    from jax.experimental.shard_map import shard_map
    from concourse import mybir
    from concourse.bass2jax import _bass_exec_p, install_neuronx_cc_hook, partition_id_tensor

    nc = _build_nc(loop_iters)
    install_neuronx_cc_hook()

    partition_name = nc.partition_id_tensor.name if nc.partition_id_tensor else None
    in_names, out_names, out_avals = [], [], []
    for alloc in nc.m.functions[0].allocations:
        if not isinstance(alloc, mybir.MemoryLocationSet):
            continue
        name = alloc.memorylocations[0].name
        if alloc.kind == "ExternalInput":
            if name != partition_name:
                in_names.append(name)
        elif alloc.kind == "ExternalOutput":
            shape = tuple(alloc.tensor_shape)
            dtype = mybir.dt.np(alloc.dtype)
            out_names.append(name)
            out_avals.append(jax.core.ShapedArray(shape, dtype))
    n_params = len(in_names)
    in_names_all = list(in_names) + out_names
    if partition_name is not None:
        in_names_all.append(partition_name)

    def _body(*args):
        operands = list(args)
        if partition_name is not None:
            operands.append(partition_id_tensor())
        outs = _bass_exec_p.bind(
            *operands,
            out_avals=tuple(out_avals),
            in_names=tuple(in_names_all),
            out_names=tuple(out_names),
            lowering_input_output_aliases=(),
            sim_require_finite=True,
            sim_require_nnan=True,
            nc=nc,
        )
        return tuple(outs)

    devices = jax.devices()[:N_CORES]
    assert len(devices) == N_CORES
    mesh = Mesh(np.asarray(devices), ("core",))
    sharding = NamedSharding(mesh, PartitionSpec("core"))
    n_outs = len(out_names)
    sharded = jax.jit(
        shard_map(
            _body,
            mesh=mesh,
            in_specs=(PartitionSpec("core"),) * (n_params + n_outs),
            out_specs=(PartitionSpec("core"),) * n_outs,
            check_rep=False,
        ),
        keep_unused=True,
    )

    def prepare(in_maps):
        """Concatenate per-core inputs (plus output zero-buffers) and move
        them to device once; safe to reuse across calls (no donation)."""
        per_core = [[np.asarray(m[name]) for name in in_names] for m in in_maps]
        concat_in = [
            np.concatenate([per_core[c][i] for c in range(N_CORES)], axis=0)
            for i in range(n_params)
        ]
        concat_in += [
            np.zeros((N_CORES * a.shape[0], *a.shape[1:]), a.dtype) for a in out_avals
        ]
        return [jax.device_put(a, sharding) for a in concat_in]

    def run_prepared(dev_args, async_out=False):
        out_arrs = sharded(*dev_args)
        if async_out:
            return out_arrs
        return [
            {
                name: np.asarray(out_arrs[i]).reshape(N_CORES, *out_avals[i].shape)[c]
                for i, name in enumerate(out_names)
            }
            for c in range(N_CORES)
        ]

    def run(in_maps, async_out=False):
        return run_prepared(prepare(in_maps), async_out=async_out)

    run.prepare = prepare
    run.run_prepared = run_prepared
    _CACHE[key] = run
    return run


def _shard_inputs(q, k, Ww, Wb, Uw, Ub, Vw):
    WwT = np.ascontiguousarray(Ww.T)
    UwTb = np.ascontiguousarray(Uw.T).astype(ml_dtypes.bfloat16)
    Wbc = np.ascontiguousarray(Wb.reshape(D, 1))
    Ubc = np.ascontiguousarray(Ub.reshape(D, 1))
    # one-hot-expanded V: VQ[d, q*TQ + j] = V[d] * (j == q)
    VQ = np.zeros((D, TQ * TQ), np.float32)
    for qq in range(TQ):
        VQ[:, qq * TQ + qq] = Vw.reshape(D)
    VQ = np.ascontiguousarray(VQ)
    in_maps = []
    for i in range(N_CORES):
        qc = q[NB * i : NB * (i + 1)]
        kc = np.ascontiguousarray(k[NB * i : NB * (i + 1)])
        in_maps.append(
            {
                "qT": np.ascontiguousarray(qc.reshape(NB * TQ, D).T),
                "k": kc,
                "kb": kc.astype(ml_dtypes.bfloat16),
                "WwT": WwT,
                "UwTb": UwTb,
                "Wb": Wbc,
                "Ub": Ubc,
                "VQ": VQ,
            }
        )
    return in_maps


def kernel(q, k, mask, Ww, Wb, Uw, Ub, Vw, Vb):
    q = np.asarray(q, np.float32)
    k = np.asarray(k, np.float32)
    # mask and Vb shift scores uniformly across the softmax (q) axis,
    # so they cancel exactly; they are not used on device.
    run = _get_runner()
    in_maps = _shard_inputs(
        q,
        k,
        np.asarray(Ww, np.float32),
        np.asarray(Wb, np.float32),
        np.asarray(Uw, np.float32),
        np.asarray(Ub, np.float32),
        np.asarray(Vw, np.float32),
    )
    res = run(in_maps)
    return np.concatenate([res[i]["out"] for i in range(N_CORES)], axis=0)
